# revision 36
# baseline (speedup 1.0000x reference)
"""Single-head attention block (Q/K/V/O projections + softmax attention) on
8 Trainium2 NeuronCores.

Problem: x [16, 2048, 512] fp32; four 512x512 projections (torch convention
y = x @ W.T + b); scores = Q @ K.T / sqrt(512); softmax over keys;
out = attn @ V; y = out @ Wo.T + bo.

Sharding: pure data-parallel over batch - each of the 8 cores computes 2 of
the 16 batches end-to-end. No collectives.

Algebraic restructuring (softmax is invariant to adding any function of the
query row, so those terms are dropped):
  scores = (x Wq^T + bq)(x Wk^T + bk)^T / sqrt(D)
         ~ x A x^T + w[k]      with A = Wq^T Wk / sqrt(D)  (precomputed once)
                                    w = x (Wk^T bq) / sqrt(D)
  out = attn (x Wv^T + bv);  y = out Wo^T + bo
      = attn x B + c          with B = Wv^T Wo^T (once), c = bv Wo^T + bo
This removes the Q, K and V projections entirely: per batch only
  HT[d',q] = A-tiles.T @ xT    (one projection instead of three)
  scoresT[k,q] = xT-tiles.T @ HT  -> exp(. + w[k]) on ACT (w rides the bias)
  ZT[d,q] += x-tiles.T @ attnT ;  rs[1,q] += ones.T @ attnT
  y[q,g] = (ZT-tiles.T @ B) * (1/rs) + c

fp8 DoubleRow acceleration: the attention-sized matmuls (scoresT and
ZT/rowsum) run as float8e4 DoubleRow matmuls, contracting 256 rows per
instruction at the same per-instruction cost as a 128-row f32r matmul -
2x fewer PE instructions for 2/3 of the work. Scaling keeps operands out
of the fp8 subnormal range and folds back out exactly:
  xT8 = fp8(16 x),  HT8 = fp8(64 H)   -> pss = 1024 scores; exp scale=1/1024
  xN8 = fp8(16 x),  at8 = fp8(exp)    -> po = 16 Z
  ones8 = 16                          -> pr = 16 rs; 1/(16 rs) * 16 Z = Z/rs
The H->fp8 and Z->y stages stay f32r (last-stage / correlated quantization
there fails the 2e-2 budget; measured numerically). Everything accumulates
fp32 in PSUM. exp never overflows (scores ~ N(0,1/9)) so the max-subtraction
is skipped.

The per-q-chunk epilogue and the next chunk's HT are spread across the
kt-pair loop so the PE never waits on the ACT exp latency; PSUM-freeing
evictions stay eager. An 11-matmul warmup burst at kernel start flips the
PE HAM clock-gate to 2.4 GHz while the first DMAs are in flight.
"""

import os
from contextlib import ExitStack

import numpy as np

import concourse.bass as bass
import concourse.tile as tile
from concourse import bacc, mybir
from concourse.bass_utils import run_bass_kernel_spmd
from concourse.masks import make_identity

N_CORES = 8
B, S, D = 16, 2048, 512
BPC = B // N_CORES  # batches per core
P = 128
ND = D // P         # 4   tiles over d/e/f dims
NS = S // P         # 16  tiles over s (= q = k) dim
NP = NS // 2        # 8   kt-pairs per chunk-k loop
QC = 512            # s/q-chunk width (PSUM bank)
NQC = S // QC       # 4
TPC = QC // P       # 4   128-tiles per chunk
SCALE = float(1.0 / np.sqrt(D))

XS = 16.0           # fp8 scale for x (both xT8 and xN8)
HS = 64.0           # fp8 scale for H

F32 = mybir.dt.float32
F32R = mybir.dt.float32r
F8 = mybir.dt.float8e4
AFT = mybir.ActivationFunctionType
ALU = mybir.AluOpType
DR = mybir.MatmulPerfMode.DoubleRow


def _emit(tc, x_ap, w_aps, b_aps, y_ap, scores_fp8=True, zt_fp8=True):
    nc = tc.nc
    MDT = F32R
    ctx = ExitStack()
    with ctx:
        # ---- pools ----
        consts = ctx.enter_context(tc.tile_pool(name="consts", bufs=1))
        stage = ctx.enter_context(tc.tile_pool(name="stage", bufs=4))
        ab_pool = ctx.enter_context(tc.tile_pool(name="ab", bufs=1))
        xt_pool = ctx.enter_context(tc.tile_pool(name="xt", bufs=2))
        xn_pool = ctx.enter_context(
            tc.tile_pool(name="xn", bufs=16 if zt_fp8 else NS + 8)
        )
        oc_pool = ctx.enter_context(tc.tile_pool(name="oc", bufs=11))
        y_pool = ctx.enter_context(tc.tile_pool(name="y", bufs=3))
        rs_pool = ctx.enter_context(tc.tile_pool(name="rs", bufs=2))
        if scores_fp8:
            xt8_pool = ctx.enter_context(tc.tile_pool(name="xt8", bufs=2))
            ht_pool = ctx.enter_context(tc.tile_pool(name="ht8", bufs=4))
        else:
            ht_pool = ctx.enter_context(tc.tile_pool(name="ht", bufs=2 * ND))
        if zt_fp8:
            xn8_pool = ctx.enter_context(tc.tile_pool(name="xn8", bufs=18))
            at_pool = ctx.enter_context(tc.tile_pool(name="at8", bufs=4))
        else:
            xn8_pool = None
            at_pool = ctx.enter_context(tc.tile_pool(name="at", bufs=4))
        ppt = ctx.enter_context(tc.tile_pool(name="ppt", bufs=3, space="PSUM"))
        ppo = ctx.enter_context(tc.tile_pool(name="ppo", bufs=4, space="PSUM"))
        ppr = ctx.enter_context(tc.tile_pool(name="ppr", bufs=1, space="PSUM"))

        def pt_tile():
            return ppt.tile([P, QC], F32, tag="ppt", name="pt")

        # ---- constants ----
        ones_bf = consts.tile([P, P], mybir.dt.bfloat16, tag="ones_bf")
        nc.vector.memset(ones_bf[:], 1.0)

        def filler(n=1):
            # bf16 no-op matmuls that keep the PE HAM activity window busy
            # through DMA-bound stretches so the clock gate stays at 2.4 GHz
            for _ in range(n):
                ps = pt_tile()
                nc.tensor.matmul(
                    ps[:, 0:P], ones_bf[:], ones_bf[:], start=True, stop=True
                )

        def ldw_filler(n=1):
            for _ in range(n):
                nc.tensor.ldweights(ones_bf[:])

        # Dense matmul burst: ~4.5us of sustained PE activity flips the PE HAM
        # clock-gate to 8/8 (2.4 GHz) while the first DMAs are in flight.
        filler(28)
        ident = consts.tile([P, P], F32, tag="ident")
        make_identity(nc, ident[:])
        ident_r = consts.tile([P, P], MDT, tag="ident_r")
        nc.vector.tensor_copy(ident_r[:], ident[:])
        ones_stage = stage.tile([P, P], F32, tag="stage", name="ones_stage")
        nc.vector.memset(ones_stage[:], 1.0)
        if zt_fp8:
            # rowsum stationary: value 16 so pr = 16*rs, cancelling po = 16*Z
            ones8 = consts.tile([P, 2 * 16], F8, tag="ones8")
            nc.vector.memset(ones8[:], XS)

            def ones8_ap():
                return ones8[:].rearrange("p (ks c) -> p ks c", ks=2)[:, :, 0:1]
        else:
            ones_col = consts.tile([P, 1], MDT, tag="ones_col")
            nc.vector.tensor_copy(ones_col[:], ones_stage[:, 0:1])
        ones_row = consts.tile([1, P], MDT, tag="ones_row")
        nc.vector.tensor_copy(ones_row[:], ones_stage[0:1, :])

        def row_to_col(row_ap, dst_ap, scale=None):
            """[1, 128] SBUF row -> [128, 1] SBUF column via PE transpose."""
            ps = pt_tile()
            nc.tensor.transpose(ps[:, 0:1], row_ap.bitcast(F32), ident[0:1, 0:1])
            if scale is None:
                nc.vector.tensor_copy(dst_ap, ps[:, 0:1])
            else:
                nc.vector.tensor_scalar_mul(dst_ap, ps[:, 0:1], scale)

        def load_bias_row(nm):
            st = stage.tile([1, D], F32, tag="stage", name="brow")
            nc.sync.dma_start(st[:], b_aps[nm][None, :])
            return st

        def load_wnat(nm):
            """Weight, natural [row, col] layout, rounded to f32r: 4 tiles."""
            tiles = []
            for rt in range(ND):
                wst = stage.tile([P, D], F32, tag="stage", name="wst")
                nc.sync.dma_start(wst[:], w_aps[nm][P * rt : P * (rt + 1), :])
                t = oc_pool.tile([P, D], MDT, tag="oc", name=f"{nm}n{rt}")
                nc.vector.tensor_copy(t[:], wst[:])
                tiles.append(t)
            return tiles

        # ---- one-time weight setup ----
        A = [ab_pool.tile([P, D], MDT, tag=f"A{j}", name=f"A{j}") for j in range(ND)]
        Bm = [ab_pool.tile([P, D], MDT, tag=f"B{j}", name=f"B{j}") for j in range(ND)]
        v_col = consts.tile([P, ND], MDT, tag="v_col")
        w_setup = {}

        def setup_part1(wq, wk):
            # A = Wq^T Wk * SCALE ;  v = (Wk^T bq) * SCALE
            bq_row = load_bias_row("bq")
            for dt_ in range(ND):
                ps = pt_tile()
                for et in range(ND):
                    nc.tensor.matmul(
                        ps[:],
                        wq[et][:, P * dt_ : P * (dt_ + 1)],
                        wk[et][:],
                        start=(et == 0),
                        stop=(et == ND - 1),
                    )
                nc.vector.tensor_scalar_mul(A[dt_][:], ps[:], SCALE)
            bq_col = consts.tile([P, ND], MDT, tag="bq_col")
            for t in range(ND):
                row_to_col(bq_row[0:1, P * t : P * (t + 1)], bq_col[:, t : t + 1])
            psv = pt_tile()
            for et in range(ND):
                nc.tensor.matmul(
                    psv[0:1, :],
                    bq_col[:, et : et + 1],
                    wk[et][:],
                    start=(et == 0),
                    stop=(et == ND - 1),
                )
            v_row = stage.tile([1, D], F32, tag="stage", name="v_row")
            nc.vector.tensor_scalar_mul(v_row[:], psv[0:1, :], SCALE)
            for t in range(ND):
                row_to_col(v_row[0:1, P * t : P * (t + 1)], v_col[:, t : t + 1])

        def setup2_items(ws):
            # B = Wv^T Wo^T ;  c = bv Wo^T + bo  (broadcast to 128 rows),
            # sliced into spreadable closures; the Wv/Wo DMAs are deferred to
            # the first slice so their staging casts can't block the DVE
            # queue ahead of the x evictions
            woT = [
                oc_pool.tile([P, D], MDT, tag="oc", name=f"WoT{j}")
                for j in range(ND)
            ]
            box = {}

            def load_vo():
                box["wv"] = load_wnat("Wv")
                box["wo"] = load_wnat("Wo")
                box["bv"] = load_bias_row("bv")
                box["bo"] = load_bias_row("bo")

            def wot_part(gts):
                wo = box["wo"]
                for gt in gts:
                    for ft in range(ND):
                        ps = pt_tile()
                        nc.tensor.transpose(
                            ps[:, 0:P],
                            wo[gt][:, P * ft : P * (ft + 1)].bitcast(F32),
                            ident[:],
                        )
                        nc.vector.tensor_copy(
                            woT[ft][:, P * gt : P * (gt + 1)], ps[:, 0:P]
                        )

            def bm_part(dts):
                wv = box["wv"]
                for dt_ in dts:
                    ps = pt_tile()
                    for ft in range(ND):
                        nc.tensor.matmul(
                            ps[:],
                            wv[ft][:, P * dt_ : P * (dt_ + 1)],
                            woT[ft][:],
                            start=(ft == 0),
                            stop=(ft == ND - 1),
                        )
                    nc.vector.tensor_copy(Bm[dt_][:], ps[:])

            def c_part():
                bv_row, bo_row = box["bv"], box["bo"]
                bv_col = stage.tile([P, ND], MDT, tag="stage", name="bv_col")
                for t in range(ND):
                    row_to_col(
                        bv_row[0:1, P * t : P * (t + 1)], bv_col[:, t : t + 1]
                    )
                psc = pt_tile()
                for ft in range(ND):
                    nc.tensor.matmul(
                        psc[0:1, :],
                        bv_col[:, ft : ft + 1],
                        woT[ft][:],
                        start=(ft == 0),
                        stop=(ft == ND - 1),
                    )
                c_row = stage.tile([1, D], MDT, tag="stage", name="c_row")
                nc.vector.tensor_add(c_row[:], psc[0:1, :], bo_row[0:1, :])
                psb = pt_tile()
                nc.tensor.matmul(psb[:], ones_row[:], c_row[:], start=True, stop=True)
                c_bc = consts.tile([P, D], F32, tag="c_bc")
                nc.vector.tensor_copy(c_bc[:], psb[:])
                w_setup["c_bc"] = c_bc

            return [
                load_vo,
                lambda: wot_part((0, 1)),
                lambda: wot_part((2, 3)),
                lambda: bm_part((0, 1)),
                lambda: bm_part((2, 3)),
                c_part,
            ]

        # per-q-chunk epilogue. The PSUM-freeing evictions (ZT chunk -> SBUF,
        # rowsum -> SBUF) are emitted immediately at chunk end; the PE-side
        # tail (1/rs transposes + y projection) is deferred and spread across
        # the next chunk's kt-pair loop so the PE never drains.
        state = {"pending": None}

        def evict_chunk(b, qc, po, pr):
            rsrow = rs_pool.tile([1, QC], F32, tag="rs", name="rsrow")
            nc.vector.tensor_copy(rsrow[:], pr[:])
            oc = [
                oc_pool.tile([P, QC], MDT, tag="oc", name="oc") for _ in range(ND)
            ]
            for dt_ in range(ND):
                if dt_ == 1:
                    nc.scalar.activation(oc[dt_][:], po[dt_][:], AFT.Copy)
                else:
                    nc.vector.tensor_copy(oc[dt_][:], po[dt_][:])
            return (b, qc, oc, rsrow)

        def epilogue_head(b, qc, oc, rsrow):
            # 1/rs column (reciprocal of 16*rs when zt_fp8; po holds 16*Z so
            # the factors cancel exactly)
            rsT = rs_pool.tile([P, TPC], F32, tag="rsT", name="rsT")
            for j in range(TPC):
                row_to_col(rsrow[0:1, P * j : P * (j + 1)], rsT[:, j : j + 1])
            rsr = rs_pool.tile([P, TPC], F32, tag="rsr", name="rsr")
            nc.vector.reciprocal(rsr[:], rsT[:])
            return rsr

        def epilogue_slice(b, qc, oc, rsr, j):
            i = TPC * qc + j
            ps = pt_tile()
            for dt_ in range(ND):
                nc.tensor.matmul(
                    ps[:],
                    oc[dt_][:, P * j : P * (j + 1)],
                    Bm[dt_][:],
                    start=(dt_ == 0),
                    stop=(dt_ == ND - 1),
                )
            ysb = y_pool.tile([P, D], F32, tag="y", name="ysb")
            nc.vector.scalar_tensor_tensor(
                ysb[:],
                ps[:],
                rsr[:, j : j + 1],
                w_setup["c_bc"][:],
                op0=ALU.mult,
                op1=ALU.add,
            )
            nc.sync.dma_start(y_ap[b, P * i : P * (i + 1), :], ysb[:])

        # ---- per batch ----
        # xT (f32r) is one flat [128, ND*S] tile per batch, d-tile-major:
        # column block dt*S + s holds x[s, dt*128+p]; feeds HT and w matmuls.
        # xT8 (when scores_fp8) packs the same data as fp8*16 in DoubleRow
        # layout [p, dpair, ksub, s] (contraction d = dpair*256 + ksub*128 + p).
        xTs = [
            xt_pool.tile([P, ND * S], MDT, tag="xt", name=f"xT{b}")
            for b in range(BPC)
        ]
        if scores_fp8:
            # per batch: flat [p, (dpair ksub s)] = fp8(16 x[s, .])
            # contraction d = dpair*256 + ksub*128 + p
            xT8s = [
                xt8_pool.tile([P, 2 * 2 * S], F8, tag="xt8", name=f"xT8{b}")
                for b in range(BPC)
            ]

            def xt8_slice(bb, dp, kt):
                return (
                    xT8s[bb][:, dp * 2 * S : (dp + 1) * 2 * S]
                    .rearrange("p (ks s) -> p ks s", ks=2)[
                        :, :, P * kt : P * (kt + 1)
                    ]
                )
        if zt_fp8:
            # per kt-pair: flat [p, (j d)] = fp8(16 x[pair*256 + j*128 + p, d])
            xN8s = [
                [
                    xn8_pool.tile([P, 2 * D], F8, tag="xn8", name=f"xN8{b}")
                    for _ in range(NP)
                ]
                for b in range(BPC)
            ]

            def xn8_slice(bb, pair, dt_):
                return xN8s[bb][pair][:].rearrange("p (ks d) -> p ks d", ks=2)[
                    :, :, P * dt_ : P * (dt_ + 1)
                ]
        else:
            xN8s = None
        xNs = [[None] * NS for _ in range(BPC)]  # fp32 staging (short-lived)
        chunks_done = [set() for _ in range(BPC)]

        def xt_slice(bb, dt_, lo, hi):
            return xTs[bb][:, dt_ * S + lo : dt_ * S + hi]

        def emit_x_dma(bb, sc, eng=None):
            # DMA one 512-wide s-chunk of batch bb into fp32 staging
            eng = eng or nc.sync
            for j in range(TPC):
                i = TPC * sc + j
                xn = xn_pool.tile([P, D], MDT, tag="xn", name="xn")
                xNs[bb][i] = xn
                eng.dma_start(
                    xn[:], x_ap[bb, P * i : P * (i + 1), :].bitcast(F32R)
                )

        def emit_x_compute(bb, sc):
            # casts + transposes + evictions for one staged s-chunk
            chunks_done[bb].add(sc)
            for j in range(TPC):
                i = TPC * sc + j
                xn = xNs[bb][i]
                if zt_fp8:
                    nc.scalar.activation(
                        xN8s[bb][i // 2][:, (i % 2) * D : (i % 2 + 1) * D],
                        xn[:].bitcast(F32),
                        AFT.Copy,
                        scale=XS,
                    )
                ps = ppt.tile([P, QC], MDT, tag="ppt", name="ptr")
                for dt_ in range(ND):
                    nc.tensor.transpose(
                        ps[:, P * dt_ : P * (dt_ + 1)],
                        xn[:, P * dt_ : P * (dt_ + 1)],
                        ident_r[:],
                    )
                nc.vector.tensor_copy(
                    xTs[bb][:].rearrange("p (dt s) -> p dt s", dt=ND)[
                        :, :, P * i : P * (i + 1)
                    ],
                    ps[:].rearrange("p (dt c) -> p dt c", dt=ND),
                )
                if scores_fp8:
                    nc.vector.tensor_scalar_mul(
                        xT8s[bb][:]
                        .rearrange("p (dp ks s) -> p dp ks s", dp=2, ks=2)[
                            :, :, :, P * i : P * (i + 1)
                        ],
                        ps[:]
                        .bitcast(F32)
                        .rearrange("p (dp ks c) -> p dp ks c", dp=2, ks=2),
                        XS,
                    )

        def emit_x_chunk(bb, sc):
            emit_x_dma(bb, sc)
            emit_x_compute(bb, sc)

        HTs = [[None] * NQC for _ in range(BPC)]
        w_cols = [None] * BPC
        w_rows = [None] * BPC
        heads_done = set()

        def emit_ht(bb, hsc, dpts=tuple(range(ND))):
            # HT for q-chunk hsc (JIT, spread across the previous chunk's
            # kt-loop so the PE stream stays dense). f32r matmuls; when
            # scores_fp8 the eviction casts 64*H to fp8 DoubleRow layout
            # [p, dpair, ksub, q] packed as 2 tiles [p, (ksub q)].
            HT = HTs[bb]
            if HT[hsc] is None:
                if scores_fp8:
                    HT[hsc] = [
                        ht_pool.tile([P, 2 * QC], F8, tag="ht", name="HT8")
                        for _ in range(2)
                    ]
                else:
                    HT[hsc] = [
                        ht_pool.tile([P, QC], MDT, tag="ht", name="HT")
                        for _ in range(ND)
                    ]
            for dpt in dpts:
                ps = pt_tile()
                for dt_ in range(ND):
                    nc.tensor.matmul(
                        ps[:],
                        A[dt_][:, P * dpt : P * (dpt + 1)],
                        xt_slice(bb, dt_, QC * hsc, QC * (hsc + 1)),
                        start=(dt_ == 0),
                        stop=(dt_ == ND - 1),
                    )
                if scores_fp8:
                    nc.scalar.activation(
                        HT[hsc][dpt // 2][
                            :, (dpt % 2) * QC : (dpt % 2 + 1) * QC
                        ],
                        ps[:],
                        AFT.Copy,
                        scale=HS,
                    )
                else:
                    nc.scalar.activation(HT[hsc][dpt][:], ps[:], AFT.Identity)

        def w_part(bb, scs):
            # w[k] = x . v for chunks scs
            if w_rows[bb] is None:
                w_rows[bb] = rs_pool.tile(
                    [1, S], F32, tag="w_row", name="w_row", bufs=1
                )
            for sc in scs:
                psw = pt_tile()
                for dt_ in range(ND):
                    nc.tensor.matmul(
                        psw[0:1, :],
                        v_col[:, dt_ : dt_ + 1],
                        xt_slice(bb, dt_, QC * sc, QC * (sc + 1)),
                        start=(dt_ == 0),
                        stop=(dt_ == ND - 1),
                    )
                nc.vector.tensor_copy(
                    w_rows[bb][0:1, QC * sc : QC * (sc + 1)], psw[0:1, :]
                )

        def wcol_part(bb, lo, hi):
            if w_cols[bb] is None:
                w_cols[bb] = rs_pool.tile([P, NS], F32, tag="w_col", name="w_col")
            for i in range(lo, hi):
                row_to_col(
                    w_rows[bb][0:1, P * i : P * (i + 1)],
                    w_cols[bb][:, i : i + 1],
                )

        def batch_head_items(bb):
            # everything batch bb needs before its first kt loop, sliced for
            # spreading across the previous batch's last two chunks
            heads_done.add(bb)
            chunk_items = [
                lambda sc=sc: emit_x_chunk(bb, sc)
                for sc in range(NQC)
                if sc not in chunks_done[bb]
            ]
            tail = [
                lambda: w_part(bb, (0, 1)),
                lambda: w_part(bb, (2, 3)),
                lambda: wcol_part(bb, 0, 8),
                lambda: wcol_part(bb, 8, 16),
            ]
            tail.extend(
                lambda dpt=dpt: emit_ht(bb, 0, [dpt]) for dpt in range(ND)
            )
            return chunk_items, tail

        exp_scale = 1.0 / (XS * HS) if scores_fp8 else 1.0
        head0_rest = []
        for b in range(BPC):
            HT = HTs[b]
            if b not in heads_done:
                # batch 0 head: fire all DMAs (Wq/Wk lead - they head the
                # A -> HT -> attention chain), fillers keep the HAM clock warm
                # until the weights land, then only chunk 0's compute + w + HT
                # stand between us and the first kt loop; chunks 1-3 are
                # processed inside qc0's pair loop as their DMAs land
                heads_done.add(b)
                wsetup = getattr(_emit, "_ws", {})
                _emit._ws = wsetup
                wsetup["wq"] = load_wnat("Wq")
                wsetup["wk"] = load_wnat("Wk")
                emit_x_dma(b, 0)
                for sc in range(1, NQC):
                    # the second HWDGE queue (ACT is idle here) - the head is
                    # DMA-issue-latency bound on the sync queue otherwise
                    emit_x_dma(b, sc, eng=nc.scalar)
                filler(32)
                setup_part1(wsetup.pop("wq"), wsetup.pop("wk"))
                emit_x_compute(b, 0)
                w_part(b, (0,))
                wcol_part(b, 0, TPC)
                emit_ht(b, 0)

                def head_sc(sc):
                    return lambda: emit_x_compute(b, sc)

                def head_w(sc):
                    def fn():
                        w_part(b, (sc,))
                        wcol_part(b, TPC * sc, TPC * (sc + 1))

                    return fn

                head0_rest = [
                    head_sc(1), head_w(1),
                    head_sc(2), head_w(2),
                    head_sc(3), head_w(3),
                ]
            for qc in range(NQC):
                po = [
                    ppo.tile([P, QC], F32, tag="ppo", name="po") for _ in range(ND)
                ]
                pr = ppr.tile([1, QC], F32, tag="ppr", name="pr")
                # software-pipelined: scoresT(kt+1) overlaps exp(kt) on ACT
                pss = [None] * NS
                at = [None] * (NP if zt_fp8 else NS)

                def scores(kt):
                    ps = pt_tile()
                    if scores_fp8:
                        for dp in range(2):
                            nc.tensor.matmul(
                                ps[:],
                                xt8_slice(b, dp, kt),
                                HT[qc][dp][:].rearrange(
                                    "p (ks q) -> p ks q", ks=2
                                ),
                                start=(dp == 0),
                                stop=(dp == 1),
                                perf_mode=DR,
                            )
                    else:
                        for dt_ in range(ND):
                            nc.tensor.matmul(
                                ps[:],
                                xt_slice(b, dt_, P * kt, P * (kt + 1)),
                                HT[qc][dt_][:],
                                start=(dt_ == 0),
                                stop=(dt_ == ND - 1),
                            )
                    pss[kt] = ps

                # deferred PE work, spread one slice per kt-pair: the PE needs
                # >~700ns of independent work between each pair's scores and
                # its ZT (which waits on the ACT exps) to never stall
                ep_head_fn = None
                ep_slices = []
                if state["pending"] is not None:
                    pb, pqc, poc, prs = state["pending"]
                    rsr_box = {}

                    def ep_head(pb=pb, pqc=pqc, poc=poc, prs=prs):
                        rsr_box["rsr"] = epilogue_head(pb, pqc, poc, prs)

                    ep_head_fn = ep_head
                    ep_slices = [
                        lambda pb=pb, pqc=pqc, poc=poc, j=j: epilogue_slice(
                            pb, pqc, poc, rsr_box["rsr"], j
                        )
                        for j in range(TPC)
                    ]
                    state["pending"] = None
                ht_items = (
                    [
                        lambda dpt=dpt, n=qc + 1: emit_ht(b, n, [dpt])
                        for dpt in range(ND)
                    ]
                    if qc + 1 < NQC
                    else []
                )
                aux_pre, aux_post = [], []
                if b + 1 < BPC:
                    if zt_fp8:
                        # next batch's head: one x-chunk morph per q-chunk
                        # (two in one kt loop oversubscribes ACT/DVE), the
                        # matmul-dense tail in the last
                        if qc == 0:
                            state["next_head"] = batch_head_items(b + 1)
                        chunks, tail = state["next_head"]
                        aux_pre = chunks[qc : qc + 1]
                        if qc == NQC - 1:
                            aux_post = tail
                            del state["next_head"]
                    elif qc == NQC - 1:
                        # f32r fallback: x tiles for 2 batches don't fit SBUF;
                        # prefetch only the first two chunks as before
                        aux_post = [
                            lambda: emit_x_chunk(b + 1, 0),
                            lambda: emit_x_chunk(b + 1, 1),
                        ]
                if b == 0 and qc == 0:
                    aux_pre = head0_rest + aux_pre
                    head0_rest = []
                    aux_post = setup2_items(_emit._ws) + aux_post
                core = [
                    x
                    for pair_ in zip(
                        ep_slices or [None] * ND, ht_items or [None] * ND
                    )
                    for x in pair_
                    if x is not None
                ]
                def filler_slice():
                    # one PSUM tile, three back-to-back no-op matmuls: PE-only
                    # padding that keeps the pair pipeline fed and HAM warm
                    ps = pt_tile()
                    for _ in range(3):
                        nc.tensor.matmul(
                            ps[:, 0:P], ones_bf[:], ones_bf[:],
                            start=True, stop=True,
                        )

                spread = aux_pre + core + aux_post
                while len(spread) < NP:  # keep the PE fed (and HAM warm)
                    spread.append(filler_slice)

                scores(0)
                if ep_head_fn is not None:
                    ep_head_fn()
                for kt in range(NS):
                    if zt_fp8:
                        pair = kt // 2
                        if kt % 2 == 0:
                            at[pair] = at_pool.tile(
                                [P, 2 * QC], F8, tag="at", name="at8"
                            )
                        nc.scalar.activation(
                            at[pair][:, (kt % 2) * QC : (kt % 2 + 1) * QC],
                            pss[kt][:],
                            AFT.Exp,
                            bias=w_cols[b][:, kt : kt + 1],
                            scale=exp_scale,
                        )
                    else:
                        a = at_pool.tile([P, QC], MDT, tag="at", name="at")
                        nc.scalar.activation(
                            a[:], pss[kt][:], AFT.Exp,
                            bias=w_cols[b][:, kt : kt + 1], scale=exp_scale,
                        )
                        at[kt] = a
                    if kt + 1 < NS:
                        scores(kt + 1)
                    if spread and kt % 2 == 1:
                        spread.pop(0)()
                    if zt_fp8:
                        if kt % 2 == 1:
                            at_mv = at[pair][:].rearrange(
                                "p (ks q) -> p ks q", ks=2
                            )
                            for dt_ in range(ND):
                                nc.tensor.matmul(
                                    po[dt_][:],
                                    xn8_slice(b, pair, dt_),
                                    at_mv,
                                    start=(pair == 0),
                                    stop=(pair == NP - 1),
                                    perf_mode=DR,
                                )
                            nc.tensor.matmul(
                                pr[:],
                                ones8_ap(),
                                at_mv,
                                start=(pair == 0),
                                stop=(pair == NP - 1),
                                perf_mode=DR,
                            )
                    else:
                        for dt_ in range(ND):
                            nc.tensor.matmul(
                                po[dt_][:],
                                xNs[b][kt][:, P * dt_ : P * (dt_ + 1)],
                                at[kt][:],
                                start=(kt == 0),
                                stop=(kt == NS - 1),
                            )
                        nc.tensor.matmul(
                            pr[:],
                            ones_col[:],
                            at[kt][:],
                            start=(kt == 0),
                            stop=(kt == NS - 1),
                        )
                for fn in spread:  # anything not yet flushed
                    fn()
                state["pending"] = evict_chunk(b, qc, po, pr)

        if state["pending"] is not None:
            pb, pqc, poc, prs = state["pending"]
            rsr = epilogue_head(pb, pqc, poc, prs)
            for j in range(TPC):
                epilogue_slice(pb, pqc, poc, rsr, j)
            state["pending"] = None


def build_program(scores_fp8=True, zt_fp8=True):
    nc = bacc.Bacc("TRN2", target_bir_lowering=False, debug=False)
    x_ap = nc.dram_tensor("x", [BPC, S, D], F32, kind="ExternalInput").ap()
    w_aps = {
        nm: nc.dram_tensor(nm, [D, D], F32, kind="ExternalInput").ap()
        for nm in ("Wq", "Wk", "Wv", "Wo")
    }
    b_aps = {
        nm: nc.dram_tensor(nm, [D], F32, kind="ExternalInput").ap()
        for nm in ("bq", "bk", "bv", "bo")
    }
    y_ap = nc.dram_tensor("y", [BPC, S, D], F32, kind="ExternalOutput").ap()
    with tile.TileContext(nc) as tc:
        _emit(tc, x_ap, w_aps, b_aps, y_ap, scores_fp8=scores_fp8, zt_fp8=zt_fp8)
    nc.compile()
    return nc


_program_cache = {}


def _get_program(scores_fp8=True, zt_fp8=True):
    key = (scores_fp8, zt_fp8)
    if key not in _program_cache:
        _program_cache[key] = build_program(scores_fp8, zt_fp8)
    return _program_cache[key]


def _make_in_maps(inputs):
    arrs = {
        k: np.ascontiguousarray(np.asarray(v, dtype=np.float32))
        for k, v in inputs.items()
    }
    in_maps = []
    for core in range(N_CORES):
        m = {"x": arrs["x"][BPC * core : BPC * (core + 1)]}
        for nm in ("Wq", "Wk", "Wv", "Wo", "bq", "bk", "bv", "bo"):
            m[nm] = arrs[nm]
        in_maps.append(m)
    return in_maps


def run(inputs, scores_fp8=None, zt_fp8=None, trace=False):
    """Returns (y_full, BassKernelResults)."""
    if scores_fp8 is None:
        scores_fp8 = os.environ.get("KERNEL_SCORES_FP8", "1") != "0"
    if zt_fp8 is None:
        zt_fp8 = os.environ.get("KERNEL_ZT_FP8", "1") != "0"
    nc = _get_program(scores_fp8, zt_fp8)
    in_maps = _make_in_maps(inputs)
    last_err = None
    for attempt in range(3):
        try:
            res = run_bass_kernel_spmd(nc, in_maps, list(range(N_CORES)), trace=trace)
            break
        except Exception as e:  # transient NRT device errors: retry
            last_err = e
            import time

            time.sleep(2.0 * (attempt + 1))
    else:
        raise last_err
    y = np.concatenate([r["y"] for r in res.results], axis=0)
    return np.ascontiguousarray(y.astype(np.float32)), res


def kernel(**inputs):
    y, _ = run(inputs, trace=False)
    return y


# revision 37
# speedup vs baseline: 1.1853x; 1.1853x over previous
"""Single-head attention block (Q/K/V/O projections + softmax attention) on
8 Trainium2 NeuronCores.

Problem: x [16, 2048, 512] fp32; four 512x512 projections (torch convention
y = x @ W.T + b); scores = Q @ K.T / sqrt(512); softmax over keys;
out = attn @ V; y = out @ Wo.T + bo.

Sharding: pure data-parallel over batch - each of the 8 cores computes 2 of
the 16 batches end-to-end. No collectives.

Algebraic restructuring (softmax is invariant to adding any function of the
query row, so those terms are dropped):
  scores = (x Wq^T + bq)(x Wk^T + bk)^T / sqrt(D)
         ~ x A x^T + w[k]      with A = Wq^T Wk / sqrt(D)  (precomputed once)
                                    w = x (Wk^T bq) / sqrt(D)
  out = attn (x Wv^T + bv);  y = out Wo^T + bo
      = attn x B + c          with B = Wv^T Wo^T (once), c = bv Wo^T + bo
This removes the Q, K and V projections entirely: per batch only
  HT[d',q] = A-tiles.T @ xT    (one projection instead of three)
  scoresT[k,q] = xT-tiles.T @ HT  -> exp(. + w[k]) on ACT (w rides the bias)
  ZT[d,q] += x-tiles.T @ attnT ;  rs[1,q] += ones.T @ attnT
  y[q,g] = (ZT-tiles.T @ B) * (1/rs) + c

fp8 DoubleRow acceleration: the attention-sized matmuls (scoresT and
ZT/rowsum) run as float8e4 DoubleRow matmuls, contracting 256 rows per
instruction at the same per-instruction cost as a 128-row f32r matmul -
2x fewer PE instructions for 2/3 of the work. Scaling keeps operands out
of the fp8 subnormal range and folds back out exactly:
  xT8 = fp8(16 x),  HT8 = fp8(64 H)   -> pss = 1024 scores; exp scale=1/1024
  xN8 = fp8(16 x),  at8 = fp8(exp)    -> po = 16 Z
  ones8 = 16                          -> pr = 16 rs; 1/(16 rs) * 16 Z = Z/rs
The H->fp8 and Z->y stages stay f32r (last-stage / correlated quantization
there fails the 2e-2 budget; measured numerically). Everything accumulates
fp32 in PSUM. exp never overflows (scores ~ N(0,1/9)) so the max-subtraction
is skipped.

The per-q-chunk epilogue and the next chunk's HT are spread across the
kt-pair loop so the PE never waits on the ACT exp latency; PSUM-freeing
evictions stay eager. An 11-matmul warmup burst at kernel start flips the
PE HAM clock-gate to 2.4 GHz while the first DMAs are in flight.
"""

import os
from contextlib import ExitStack

import numpy as np

import concourse.bass as bass
import concourse.tile as tile
from concourse import bacc, mybir
from concourse.bass_utils import run_bass_kernel_spmd
from concourse.masks import make_identity

N_CORES = 8
B, S, D = 16, 2048, 512
BPC = B // N_CORES  # batches per core
P = 128
ND = D // P         # 4   tiles over d/e/f dims
NS = S // P         # 16  tiles over s (= q = k) dim
NP = NS // 2        # 8   kt-pairs per chunk-k loop
QC = 512            # s/q-chunk width (PSUM bank)
NQC = S // QC       # 4
TPC = QC // P       # 4   128-tiles per chunk
SCALE = float(1.0 / np.sqrt(D))

XS = 16.0           # fp8 scale for x (both xT8 and xN8)
HS = 64.0           # fp8 scale for H

F32 = mybir.dt.float32
F32R = mybir.dt.float32r
F8 = mybir.dt.float8e4
AFT = mybir.ActivationFunctionType
ALU = mybir.AluOpType
DR = mybir.MatmulPerfMode.DoubleRow


def _emit(tc, x_ap, w_aps, b_aps, y_ap, scores_fp8=True, zt_fp8=True):
    nc = tc.nc
    MDT = F32R
    ctx = ExitStack()
    with ctx:
        # ---- pools ----
        consts = ctx.enter_context(tc.tile_pool(name="consts", bufs=1))
        stage = ctx.enter_context(tc.tile_pool(name="stage", bufs=4))
        ab_pool = ctx.enter_context(tc.tile_pool(name="ab", bufs=1))
        xt_pool = ctx.enter_context(tc.tile_pool(name="xt", bufs=2))
        xn_pool = ctx.enter_context(
            tc.tile_pool(name="xn", bufs=16 if zt_fp8 else NS + 8)
        )
        oc_pool = ctx.enter_context(tc.tile_pool(name="oc", bufs=11))
        y_pool = ctx.enter_context(tc.tile_pool(name="y", bufs=3))
        rs_pool = ctx.enter_context(tc.tile_pool(name="rs", bufs=2))
        if scores_fp8:
            xt8_pool = ctx.enter_context(tc.tile_pool(name="xt8", bufs=2))
            ht_pool = ctx.enter_context(tc.tile_pool(name="ht8", bufs=4))
        else:
            ht_pool = ctx.enter_context(tc.tile_pool(name="ht", bufs=2 * ND))
        if zt_fp8:
            xn8_pool = ctx.enter_context(tc.tile_pool(name="xn8", bufs=18))
            at_pool = ctx.enter_context(tc.tile_pool(name="at8", bufs=4))
        else:
            xn8_pool = None
            at_pool = ctx.enter_context(tc.tile_pool(name="at", bufs=4))
        ppt = ctx.enter_context(tc.tile_pool(name="ppt", bufs=3, space="PSUM"))
        ppo = ctx.enter_context(tc.tile_pool(name="ppo", bufs=4, space="PSUM"))
        ppr = ctx.enter_context(tc.tile_pool(name="ppr", bufs=1, space="PSUM"))

        def pt_tile():
            return ppt.tile([P, QC], F32, tag="ppt", name="pt")

        # ---- constants ----
        ones_bf = consts.tile([P, P], mybir.dt.bfloat16, tag="ones_bf")
        nc.vector.memset(ones_bf[:], 1.0)

        def filler(n=1):
            # bf16 no-op matmuls that keep the PE HAM activity window busy
            # through DMA-bound stretches so the clock gate stays at 2.4 GHz
            for _ in range(n):
                ps = pt_tile()
                nc.tensor.matmul(
                    ps[:, 0:P], ones_bf[:], ones_bf[:], start=True, stop=True
                )

        def ldw_filler(n=1):
            for _ in range(n):
                nc.tensor.ldweights(ones_bf[:])

        # Dense matmul burst: ~4.5us of sustained PE activity flips the PE HAM
        # clock-gate to 8/8 (2.4 GHz) while the first DMAs are in flight.
        filler(28)
        ident = consts.tile([P, P], F32, tag="ident")
        make_identity(nc, ident[:])
        ident_r = consts.tile([P, P], MDT, tag="ident_r")
        nc.vector.tensor_copy(ident_r[:], ident[:])
        ones_stage = stage.tile([P, P], F32, tag="stage", name="ones_stage")
        nc.vector.memset(ones_stage[:], 1.0)
        if zt_fp8:
            # rowsum stationary: value 16 so pr = 16*rs, cancelling po = 16*Z
            ones8 = consts.tile([P, 2 * 16], F8, tag="ones8")
            nc.vector.memset(ones8[:], XS)

            def ones8_ap():
                return ones8[:].rearrange("p (ks c) -> p ks c", ks=2)[:, :, 0:1]
        else:
            ones_col = consts.tile([P, 1], MDT, tag="ones_col")
            nc.vector.tensor_copy(ones_col[:], ones_stage[:, 0:1])
        ones_row = consts.tile([1, P], MDT, tag="ones_row")
        nc.vector.tensor_copy(ones_row[:], ones_stage[0:1, :])

        def row_to_col(row_ap, dst_ap, scale=None):
            """[1, 128] SBUF row -> [128, 1] SBUF column via PE transpose."""
            ps = pt_tile()
            nc.tensor.transpose(ps[:, 0:1], row_ap.bitcast(F32), ident[0:1, 0:1])
            if scale is None:
                nc.vector.tensor_copy(dst_ap, ps[:, 0:1])
            else:
                nc.vector.tensor_scalar_mul(dst_ap, ps[:, 0:1], scale)

        def load_bias_row(nm):
            st = stage.tile([1, D], F32, tag="stage", name="brow")
            nc.sync.dma_start(st[:], b_aps[nm][None, :])
            return st

        def load_wnat(nm):
            """Weight, natural [row, col] layout, rounded to f32r: 4 tiles."""
            tiles = []
            for rt in range(ND):
                wst = stage.tile([P, D], F32, tag="stage", name="wst")
                nc.sync.dma_start(wst[:], w_aps[nm][P * rt : P * (rt + 1), :])
                t = oc_pool.tile([P, D], MDT, tag="oc", name=f"{nm}n{rt}")
                nc.vector.tensor_copy(t[:], wst[:])
                tiles.append(t)
            return tiles

        # ---- one-time weight setup ----
        A = [ab_pool.tile([P, D], MDT, tag=f"A{j}", name=f"A{j}") for j in range(ND)]
        Bm = [ab_pool.tile([P, D], MDT, tag=f"B{j}", name=f"B{j}") for j in range(ND)]
        v_col = consts.tile([P, ND], MDT, tag="v_col")
        w_setup = {}

        def setup_part1(wq, wk):
            # A = Wq^T Wk * SCALE ;  v = (Wk^T bq) * SCALE
            bq_row = load_bias_row("bq")
            for dt_ in range(ND):
                ps = pt_tile()
                for et in range(ND):
                    nc.tensor.matmul(
                        ps[:],
                        wq[et][:, P * dt_ : P * (dt_ + 1)],
                        wk[et][:],
                        start=(et == 0),
                        stop=(et == ND - 1),
                    )
                nc.vector.tensor_scalar_mul(A[dt_][:], ps[:], SCALE)
            bq_col = consts.tile([P, ND], MDT, tag="bq_col")
            for t in range(ND):
                row_to_col(bq_row[0:1, P * t : P * (t + 1)], bq_col[:, t : t + 1])
            psv = pt_tile()
            for et in range(ND):
                nc.tensor.matmul(
                    psv[0:1, :],
                    bq_col[:, et : et + 1],
                    wk[et][:],
                    start=(et == 0),
                    stop=(et == ND - 1),
                )
            v_row = stage.tile([1, D], F32, tag="stage", name="v_row")
            nc.vector.tensor_scalar_mul(v_row[:], psv[0:1, :], SCALE)
            for t in range(ND):
                row_to_col(v_row[0:1, P * t : P * (t + 1)], v_col[:, t : t + 1])

        def setup2_items(ws):
            # B = Wv^T Wo^T ;  c = bv Wo^T + bo  (broadcast to 128 rows),
            # sliced into spreadable closures; the Wv/Wo DMAs are deferred to
            # the first slice so their staging casts can't block the DVE
            # queue ahead of the x evictions
            woT = [
                oc_pool.tile([P, D], MDT, tag="oc", name=f"WoT{j}")
                for j in range(ND)
            ]
            box = {}

            def load_vo():
                box["wv"] = load_wnat("Wv")
                box["wo"] = load_wnat("Wo")
                box["bv"] = load_bias_row("bv")
                box["bo"] = load_bias_row("bo")

            def wot_part(gts):
                wo = box["wo"]
                for gt in gts:
                    for ft in range(ND):
                        ps = pt_tile()
                        nc.tensor.transpose(
                            ps[:, 0:P],
                            wo[gt][:, P * ft : P * (ft + 1)].bitcast(F32),
                            ident[:],
                        )
                        nc.vector.tensor_copy(
                            woT[ft][:, P * gt : P * (gt + 1)], ps[:, 0:P]
                        )

            def bm_part(dts):
                wv = box["wv"]
                for dt_ in dts:
                    ps = pt_tile()
                    for ft in range(ND):
                        nc.tensor.matmul(
                            ps[:],
                            wv[ft][:, P * dt_ : P * (dt_ + 1)],
                            woT[ft][:],
                            start=(ft == 0),
                            stop=(ft == ND - 1),
                        )
                    nc.vector.tensor_copy(Bm[dt_][:], ps[:])

            def c_part():
                bv_row, bo_row = box["bv"], box["bo"]
                bv_col = stage.tile([P, ND], MDT, tag="stage", name="bv_col")
                for t in range(ND):
                    row_to_col(
                        bv_row[0:1, P * t : P * (t + 1)], bv_col[:, t : t + 1]
                    )
                psc = pt_tile()
                for ft in range(ND):
                    nc.tensor.matmul(
                        psc[0:1, :],
                        bv_col[:, ft : ft + 1],
                        woT[ft][:],
                        start=(ft == 0),
                        stop=(ft == ND - 1),
                    )
                c_row = stage.tile([1, D], MDT, tag="stage", name="c_row")
                nc.vector.tensor_add(c_row[:], psc[0:1, :], bo_row[0:1, :])
                psb = pt_tile()
                nc.tensor.matmul(psb[:], ones_row[:], c_row[:], start=True, stop=True)
                c_bc = consts.tile([P, D], F32, tag="c_bc")
                nc.vector.tensor_copy(c_bc[:], psb[:])
                w_setup["c_bc"] = c_bc

            return [
                load_vo,
                lambda: wot_part((0, 1)),
                lambda: wot_part((2, 3)),
                lambda: bm_part((0, 1)),
                lambda: bm_part((2, 3)),
                c_part,
            ]

        # per-q-chunk epilogue. The PSUM-freeing evictions (ZT chunk -> SBUF,
        # rowsum -> SBUF) are emitted immediately at chunk end; the PE-side
        # tail (1/rs transposes + y projection) is deferred and spread across
        # the next chunk's kt-pair loop so the PE never drains.
        state = {"pending": None}

        def evict_chunk(b, qc, po, pr):
            rsrow = rs_pool.tile([1, QC], F32, tag="rs", name="rsrow")
            nc.vector.tensor_copy(rsrow[:], pr[:])
            oc = [
                oc_pool.tile([P, QC], MDT, tag="oc", name="oc") for _ in range(ND)
            ]
            for dt_ in range(ND):
                if dt_ == 1:
                    nc.scalar.activation(oc[dt_][:], po[dt_][:], AFT.Copy)
                else:
                    nc.vector.tensor_copy(oc[dt_][:], po[dt_][:])
            return (b, qc, oc, rsrow)

        def epilogue_head(b, qc, oc, rsrow):
            # 1/rs column (reciprocal of 16*rs when zt_fp8; po holds 16*Z so
            # the factors cancel exactly)
            rsT = rs_pool.tile([P, TPC], F32, tag="rsT", name="rsT")
            for j in range(TPC):
                row_to_col(rsrow[0:1, P * j : P * (j + 1)], rsT[:, j : j + 1])
            rsr = rs_pool.tile([P, TPC], F32, tag="rsr", name="rsr")
            nc.vector.reciprocal(rsr[:], rsT[:])
            return rsr

        def epilogue_slice(b, qc, oc, rsr, j):
            i = TPC * qc + j
            ps = pt_tile()
            for dt_ in range(ND):
                nc.tensor.matmul(
                    ps[:],
                    oc[dt_][:, P * j : P * (j + 1)],
                    Bm[dt_][:],
                    start=(dt_ == 0),
                    stop=(dt_ == ND - 1),
                )
            ysb = y_pool.tile([P, D], F32, tag="y", name="ysb")
            nc.vector.scalar_tensor_tensor(
                ysb[:],
                ps[:],
                rsr[:, j : j + 1],
                w_setup["c_bc"][:],
                op0=ALU.mult,
                op1=ALU.add,
            )
            nc.sync.dma_start(y_ap[b, P * i : P * (i + 1), :], ysb[:])

        # ---- per batch ----
        # xT (f32r) is one flat [128, ND*S] tile per batch, d-tile-major:
        # column block dt*S + s holds x[s, dt*128+p]; feeds HT and w matmuls.
        # xT8 (when scores_fp8) packs the same data as fp8*16 in DoubleRow
        # layout [p, dpair, ksub, s] (contraction d = dpair*256 + ksub*128 + p).
        xTs = [
            xt_pool.tile([P, ND * S], MDT, tag="xt", name=f"xT{b}")
            for b in range(BPC)
        ]
        if scores_fp8:
            # per batch: flat [p, (dpair ksub s)] = fp8(16 x[s, .])
            # contraction d = dpair*256 + ksub*128 + p
            xT8s = [
                xt8_pool.tile([P, 2 * 2 * S], F8, tag="xt8", name=f"xT8{b}")
                for b in range(BPC)
            ]

            def xt8_slice(bb, dp, kt):
                return (
                    xT8s[bb][:, dp * 2 * S : (dp + 1) * 2 * S]
                    .rearrange("p (ks s) -> p ks s", ks=2)[
                        :, :, P * kt : P * (kt + 1)
                    ]
                )
        if zt_fp8:
            # per kt-pair: flat [p, (j d)] = fp8(16 x[pair*256 + j*128 + p, d])
            xN8s = [
                [
                    xn8_pool.tile([P, 2 * D], F8, tag="xn8", name=f"xN8{b}")
                    for _ in range(NP)
                ]
                for b in range(BPC)
            ]

            def xn8_slice(bb, pair, dt_):
                return xN8s[bb][pair][:].rearrange("p (ks d) -> p ks d", ks=2)[
                    :, :, P * dt_ : P * (dt_ + 1)
                ]
        else:
            xN8s = None
        xNs = [[None] * NS for _ in range(BPC)]  # fp32 staging (short-lived)
        chunks_done = [set() for _ in range(BPC)]

        def xt_slice(bb, dt_, lo, hi):
            return xTs[bb][:, dt_ * S + lo : dt_ * S + hi]

        def emit_x_dma(bb, sc, eng=None):
            # DMA one 512-wide s-chunk of batch bb into fp32 staging
            eng = eng or nc.sync
            for j in range(TPC):
                i = TPC * sc + j
                xn = xn_pool.tile([P, D], MDT, tag="xn", name="xn")
                xNs[bb][i] = xn
                eng.dma_start(
                    xn[:], x_ap[bb, P * i : P * (i + 1), :].bitcast(F32R)
                )

        def emit_x_compute(bb, sc):
            # casts + transposes + evictions for one staged s-chunk
            chunks_done[bb].add(sc)
            for j in range(TPC):
                i = TPC * sc + j
                xn = xNs[bb][i]
                if zt_fp8:
                    nc.vector.tensor_scalar_mul(
                        xN8s[bb][i // 2][:, (i % 2) * D : (i % 2 + 1) * D],
                        xn[:].bitcast(F32),
                        XS,
                    )
                ps = ppt.tile([P, QC], MDT, tag="ppt", name="ptr")
                for dt_ in range(ND):
                    nc.tensor.transpose(
                        ps[:, P * dt_ : P * (dt_ + 1)],
                        xn[:, P * dt_ : P * (dt_ + 1)],
                        ident_r[:],
                    )
                nc.vector.tensor_copy(
                    xTs[bb][:].rearrange("p (dt s) -> p dt s", dt=ND)[
                        :, :, P * i : P * (i + 1)
                    ],
                    ps[:].rearrange("p (dt c) -> p dt c", dt=ND),
                )
                if scores_fp8:
                    nc.vector.tensor_scalar_mul(
                        xT8s[bb][:]
                        .rearrange("p (dp ks s) -> p dp ks s", dp=2, ks=2)[
                            :, :, :, P * i : P * (i + 1)
                        ],
                        ps[:]
                        .bitcast(F32)
                        .rearrange("p (dp ks c) -> p dp ks c", dp=2, ks=2),
                        XS,
                    )

        def emit_x_chunk(bb, sc):
            emit_x_dma(bb, sc)
            emit_x_compute(bb, sc)

        HTs = [[None] * NQC for _ in range(BPC)]
        w_cols = [None] * BPC
        w_rows = [None] * BPC
        heads_done = set()

        def emit_ht(bb, hsc, dpts=tuple(range(ND))):
            # HT for q-chunk hsc (JIT, spread across the previous chunk's
            # kt-loop so the PE stream stays dense). f32r matmuls; when
            # scores_fp8 the eviction casts 64*H to fp8 DoubleRow layout
            # [p, dpair, ksub, q] packed as 2 tiles [p, (ksub q)].
            HT = HTs[bb]
            if HT[hsc] is None:
                if scores_fp8:
                    HT[hsc] = [
                        ht_pool.tile([P, 2 * QC], F8, tag="ht", name="HT8")
                        for _ in range(2)
                    ]
                else:
                    HT[hsc] = [
                        ht_pool.tile([P, QC], MDT, tag="ht", name="HT")
                        for _ in range(ND)
                    ]
            for dpt in dpts:
                ps = pt_tile()
                for dt_ in range(ND):
                    nc.tensor.matmul(
                        ps[:],
                        A[dt_][:, P * dpt : P * (dpt + 1)],
                        xt_slice(bb, dt_, QC * hsc, QC * (hsc + 1)),
                        start=(dt_ == 0),
                        stop=(dt_ == ND - 1),
                    )
                if scores_fp8:
                    nc.scalar.activation(
                        HT[hsc][dpt // 2][
                            :, (dpt % 2) * QC : (dpt % 2 + 1) * QC
                        ],
                        ps[:],
                        AFT.Copy,
                        scale=HS,
                    )
                else:
                    nc.scalar.activation(HT[hsc][dpt][:], ps[:], AFT.Identity)

        def w_part(bb, scs):
            # w[k] = x . v for chunks scs
            if w_rows[bb] is None:
                w_rows[bb] = rs_pool.tile(
                    [1, S], F32, tag="w_row", name="w_row", bufs=1
                )
            for sc in scs:
                psw = pt_tile()
                for dt_ in range(ND):
                    nc.tensor.matmul(
                        psw[0:1, :],
                        v_col[:, dt_ : dt_ + 1],
                        xt_slice(bb, dt_, QC * sc, QC * (sc + 1)),
                        start=(dt_ == 0),
                        stop=(dt_ == ND - 1),
                    )
                nc.vector.tensor_copy(
                    w_rows[bb][0:1, QC * sc : QC * (sc + 1)], psw[0:1, :]
                )

        def wcol_part(bb, lo, hi):
            if w_cols[bb] is None:
                w_cols[bb] = rs_pool.tile([P, NS], F32, tag="w_col", name="w_col")
            for i in range(lo, hi):
                row_to_col(
                    w_rows[bb][0:1, P * i : P * (i + 1)],
                    w_cols[bb][:, i : i + 1],
                )

        def batch_head_items(bb):
            # everything batch bb needs before its first kt loop, sliced for
            # spreading across the previous batch's last two chunks
            heads_done.add(bb)
            chunk_items = [
                lambda sc=sc: emit_x_chunk(bb, sc)
                for sc in range(NQC)
                if sc not in chunks_done[bb]
            ]
            tail = [
                lambda: w_part(bb, (0, 1)),
                lambda: w_part(bb, (2, 3)),
                lambda: wcol_part(bb, 0, 8),
                lambda: wcol_part(bb, 8, 16),
            ]
            tail.extend(
                lambda dpt=dpt: emit_ht(bb, 0, [dpt]) for dpt in range(ND)
            )
            return chunk_items, tail

        exp_scale = 1.0 / (XS * HS) if scores_fp8 else 1.0
        head0_rest = []
        for b in range(BPC):
            HT = HTs[b]
            if b not in heads_done:
                # batch 0 head: fire all DMAs (Wq/Wk lead - they head the
                # A -> HT -> attention chain), fillers keep the HAM clock warm
                # until the weights land, then only chunk 0's compute + w + HT
                # stand between us and the first kt loop; chunks 1-3 are
                # processed inside qc0's pair loop as their DMAs land
                heads_done.add(b)
                wsetup = getattr(_emit, "_ws", {})
                _emit._ws = wsetup
                wsetup["wq"] = load_wnat("Wq")
                wsetup["wk"] = load_wnat("Wk")
                emit_x_dma(b, 0)
                for sc in range(1, NQC):
                    # the second HWDGE queue (ACT is idle here) - the head is
                    # DMA-issue-latency bound on the sync queue otherwise
                    emit_x_dma(b, sc, eng=nc.scalar)
                filler(32)
                setup_part1(wsetup.pop("wq"), wsetup.pop("wk"))
                emit_x_compute(b, 0)
                w_part(b, (0,))
                wcol_part(b, 0, TPC)
                emit_ht(b, 0)

                def head_sc(sc):
                    return lambda: emit_x_compute(b, sc)

                def head_w(sc):
                    def fn():
                        w_part(b, (sc,))
                        wcol_part(b, TPC * sc, TPC * (sc + 1))

                    return fn

                head0_rest = [
                    head_sc(1), head_w(1),
                    head_sc(2), head_w(2),
                    head_sc(3), head_w(3),
                ]
            for qc in range(NQC):
                po = [
                    ppo.tile([P, QC], F32, tag="ppo", name="po") for _ in range(ND)
                ]
                pr = ppr.tile([1, QC], F32, tag="ppr", name="pr")
                # software-pipelined: scoresT(kt+1) overlaps exp(kt) on ACT
                pss = [None] * NS
                at = [None] * (NP if zt_fp8 else NS)

                def scores(kt):
                    ps = pt_tile()
                    if scores_fp8:
                        for dp in range(2):
                            nc.tensor.matmul(
                                ps[:],
                                xt8_slice(b, dp, kt),
                                HT[qc][dp][:].rearrange(
                                    "p (ks q) -> p ks q", ks=2
                                ),
                                start=(dp == 0),
                                stop=(dp == 1),
                                perf_mode=DR,
                            )
                    else:
                        for dt_ in range(ND):
                            nc.tensor.matmul(
                                ps[:],
                                xt_slice(b, dt_, P * kt, P * (kt + 1)),
                                HT[qc][dt_][:],
                                start=(dt_ == 0),
                                stop=(dt_ == ND - 1),
                            )
                    pss[kt] = ps

                # deferred PE work, spread one slice per kt-pair: the PE needs
                # >~700ns of independent work between each pair's scores and
                # its ZT (which waits on the ACT exps) to never stall
                ep_head_fn = None
                ep_slices = []
                if state["pending"] is not None:
                    pb, pqc, poc, prs = state["pending"]
                    rsr_box = {}

                    def ep_head(pb=pb, pqc=pqc, poc=poc, prs=prs):
                        rsr_box["rsr"] = epilogue_head(pb, pqc, poc, prs)

                    ep_head_fn = ep_head
                    ep_slices = [
                        lambda pb=pb, pqc=pqc, poc=poc, j=j: epilogue_slice(
                            pb, pqc, poc, rsr_box["rsr"], j
                        )
                        for j in range(TPC)
                    ]
                    state["pending"] = None
                ht_items = (
                    [
                        lambda dpt=dpt, n=qc + 1: emit_ht(b, n, [dpt])
                        for dpt in range(ND)
                    ]
                    if qc + 1 < NQC
                    else []
                )
                aux_pre, aux_post = [], []
                if b + 1 < BPC:
                    if zt_fp8:
                        # next batch's head: one x-chunk morph per q-chunk
                        # (two in one kt loop oversubscribes ACT/DVE), the
                        # matmul-dense tail in the last
                        if qc == 0:
                            state["next_head"] = batch_head_items(b + 1)
                        chunks, tail = state["next_head"]
                        aux_pre = chunks[qc : qc + 1]
                        if qc == NQC - 1:
                            aux_post = tail
                            del state["next_head"]
                    elif qc == NQC - 1:
                        # f32r fallback: x tiles for 2 batches don't fit SBUF;
                        # prefetch only the first two chunks as before
                        aux_post = [
                            lambda: emit_x_chunk(b + 1, 0),
                            lambda: emit_x_chunk(b + 1, 1),
                        ]
                if b == 0 and qc == 0:
                    aux_pre = head0_rest + aux_pre
                    head0_rest = []
                    aux_post = setup2_items(_emit._ws) + aux_post
                core = [
                    x
                    for pair_ in zip(
                        ep_slices or [None] * ND, ht_items or [None] * ND
                    )
                    for x in pair_
                    if x is not None
                ]
                def filler_slice():
                    # one PSUM tile, three back-to-back no-op matmuls: PE-only
                    # padding that keeps the pair pipeline fed and HAM warm
                    ps = pt_tile()
                    for _ in range(3):
                        nc.tensor.matmul(
                            ps[:, 0:P], ones_bf[:], ones_bf[:],
                            start=True, stop=True,
                        )

                spread = aux_pre + core + aux_post
                while len(spread) < NP:  # keep the PE fed (and HAM warm)
                    spread.append(filler_slice)

                scores(0)
                if ep_head_fn is not None:
                    ep_head_fn()
                for kt in range(NS):
                    if zt_fp8:
                        pair = kt // 2
                        if kt % 2 == 0:
                            at[pair] = at_pool.tile(
                                [P, 2 * QC], F8, tag="at", name="at8"
                            )
                        nc.scalar.activation(
                            at[pair][:, (kt % 2) * QC : (kt % 2 + 1) * QC],
                            pss[kt][:],
                            AFT.Exp,
                            bias=w_cols[b][:, kt : kt + 1],
                            scale=exp_scale,
                        )
                    else:
                        a = at_pool.tile([P, QC], MDT, tag="at", name="at")
                        nc.scalar.activation(
                            a[:], pss[kt][:], AFT.Exp,
                            bias=w_cols[b][:, kt : kt + 1], scale=exp_scale,
                        )
                        at[kt] = a
                    if kt + 1 < NS:
                        scores(kt + 1)
                    if spread and kt % 2 == 1:
                        spread.pop(0)()
                    if zt_fp8:
                        if kt % 2 == 1:
                            at_mv = at[pair][:].rearrange(
                                "p (ks q) -> p ks q", ks=2
                            )
                            for dt_ in range(ND):
                                nc.tensor.matmul(
                                    po[dt_][:],
                                    xn8_slice(b, pair, dt_),
                                    at_mv,
                                    start=(pair == 0),
                                    stop=(pair == NP - 1),
                                    perf_mode=DR,
                                )
                            nc.tensor.matmul(
                                pr[:],
                                ones8_ap(),
                                at_mv,
                                start=(pair == 0),
                                stop=(pair == NP - 1),
                                perf_mode=DR,
                            )
                    else:
                        for dt_ in range(ND):
                            nc.tensor.matmul(
                                po[dt_][:],
                                xNs[b][kt][:, P * dt_ : P * (dt_ + 1)],
                                at[kt][:],
                                start=(kt == 0),
                                stop=(kt == NS - 1),
                            )
                        nc.tensor.matmul(
                            pr[:],
                            ones_col[:],
                            at[kt][:],
                            start=(kt == 0),
                            stop=(kt == NS - 1),
                        )
                for fn in spread:  # anything not yet flushed
                    fn()
                state["pending"] = evict_chunk(b, qc, po, pr)

        if state["pending"] is not None:
            pb, pqc, poc, prs = state["pending"]
            rsr = epilogue_head(pb, pqc, poc, prs)
            for j in range(TPC):
                epilogue_slice(pb, pqc, poc, rsr, j)
            state["pending"] = None


def build_program(scores_fp8=True, zt_fp8=True):
    nc = bacc.Bacc("TRN2", target_bir_lowering=False, debug=False)
    x_ap = nc.dram_tensor("x", [BPC, S, D], F32, kind="ExternalInput").ap()
    w_aps = {
        nm: nc.dram_tensor(nm, [D, D], F32, kind="ExternalInput").ap()
        for nm in ("Wq", "Wk", "Wv", "Wo")
    }
    b_aps = {
        nm: nc.dram_tensor(nm, [D], F32, kind="ExternalInput").ap()
        for nm in ("bq", "bk", "bv", "bo")
    }
    y_ap = nc.dram_tensor("y", [BPC, S, D], F32, kind="ExternalOutput").ap()
    with tile.TileContext(nc) as tc:
        _emit(tc, x_ap, w_aps, b_aps, y_ap, scores_fp8=scores_fp8, zt_fp8=zt_fp8)
    nc.compile()
    return nc


_program_cache = {}


def _get_program(scores_fp8=True, zt_fp8=True):
    key = (scores_fp8, zt_fp8)
    if key not in _program_cache:
        _program_cache[key] = build_program(scores_fp8, zt_fp8)
    return _program_cache[key]


def _make_in_maps(inputs):
    arrs = {
        k: np.ascontiguousarray(np.asarray(v, dtype=np.float32))
        for k, v in inputs.items()
    }
    in_maps = []
    for core in range(N_CORES):
        m = {"x": arrs["x"][BPC * core : BPC * (core + 1)]}
        for nm in ("Wq", "Wk", "Wv", "Wo", "bq", "bk", "bv", "bo"):
            m[nm] = arrs[nm]
        in_maps.append(m)
    return in_maps


def run(inputs, scores_fp8=None, zt_fp8=None, trace=False):
    """Returns (y_full, BassKernelResults)."""
    if scores_fp8 is None:
        scores_fp8 = os.environ.get("KERNEL_SCORES_FP8", "1") != "0"
    if zt_fp8 is None:
        zt_fp8 = os.environ.get("KERNEL_ZT_FP8", "1") != "0"
    nc = _get_program(scores_fp8, zt_fp8)
    in_maps = _make_in_maps(inputs)
    last_err = None
    for attempt in range(3):
        try:
            res = run_bass_kernel_spmd(nc, in_maps, list(range(N_CORES)), trace=trace)
            break
        except Exception as e:  # transient NRT device errors: retry
            last_err = e
            import time

            time.sleep(2.0 * (attempt + 1))
    else:
        raise last_err
    y = np.concatenate([r["y"] for r in res.results], axis=0)
    return np.ascontiguousarray(y.astype(np.float32)), res


def kernel(**inputs):
    y, _ = run(inputs, trace=False)
    return y
